# revision 6
# baseline (speedup 1.0000x reference)
"""Trainium2 Bass kernel for nn_AdvancedGCN (GCN -> GAT -> EdgeConv -> GIN ->
global-attention pooling) over N=50000 nodes / E=800000 edges, SPMD on 8
NeuronCores.

Strategy (v3): nodes are sharded 6250/core (padded to 6272 = 49 blocks of 128)
and sorted by in-degree so each 128-node block pads its in-edge list to the
block max degree.  All graph index work happens on host and is baked into int32
gather-index tables; the device program is pure dense compute.

v3 changes vs v2:
 - the GCN stage gathers a host-precomputed replicated z = (dinv*x)@gcn_W
   table in BF16 (stage 1 loses its matmul+transpose, folds run in DVE 2x
   mode, and precision improves vs the old fp8 y table),
 - stage-1's three matmuls (gat_W / B_src / B_dst) fuse into one 136-wide
   matmul; stage-2's W1b / W1d fuse into one 256-wide matmul,
 - all small weights ship in two packed tensors (2 DMAs instead of ~25);
   index tables load first so the first gather issues at ~2us,
 - EdgeConv max-aggregation writes per-q-group reduces into one wide SBUF
   tile and finishes with a single strided reduce (drops the per-group
   max-accumulate chain and the -1e30 memset),
 - the GAT softmax padding correction is computed once for all blocks,
 - DVE/Pool work (folds, weightings, EdgeConv adds) is assigned per-block by
   a cost-model ledger that also charges Pool for SWDGE descriptor
   generation of the indirect gathers,
 - final pooling AllReduce becomes AllGather + local reduce (cheaper in the
   collective cost model), deeper PSUM pools.
"""
import os
import sys

import numpy as np
import ml_dtypes

for _p in ("/opt/trn_rl_repo", "/root/.axon_site/_ro/trn_rl_repo"):
    if os.path.isdir(_p) and _p not in sys.path:
        sys.path.insert(0, _p)

try:  # persistent XLA executable cache: identical programs skip neuronxcc
    import jax
    jax.config.update("jax_compilation_cache_dir", "/tmp/jaxcache_gnn")
    jax.config.update("jax_persistent_cache_min_entry_size_bytes", -1)
    jax.config.update("jax_persistent_cache_min_compile_time_secs", 0)
except Exception:
    pass

import concourse.bass as bass
import concourse.bacc as bacc
import concourse.tile as tile
import concourse.mybir as mybir
from concourse.bass_utils import run_bass_kernel_spmd
from concourse.masks import make_identity

N, E, IN, H, G, OUT = 50000, 800000, 128, 128, 64, 10
HEADS, C = 4, 32
R = 8                    # cores
NPC = N // R             # 6250 nodes per core
NB = (NPC + 127) // 128  # 49 blocks per core
NPCP = NB * 128          # 6272 padded nodes per core
TABR = R * NPCP          # replicated table rows
GW = 132                 # g-table row width (128 g + 4 a_src)
ASENT = -15.5            # fp8-e3m4 min: sentinel a_src / u value
GTGT = 112               # target gather-group K-sum, fp8 tables
GTGT1 = 112              # target gather-group K-sum, z table
f32, i32 = mybir.dt.float32, mybir.dt.int32
bf16 = mybir.dt.bfloat16
fp8 = mybir.dt.float8e3
np_fp8 = ml_dtypes.float8_e3m4
np_bf16 = ml_dtypes.bfloat16
AF = mybir.ActivationFunctionType
OP = mybir.AluOpType
REPL = [list(range(R))]

# packed-weight layouts: (name, cols); bf16 pack and f32 pack
WPACK_BF = [("gat_cat", IN + 2 * HEADS), ("W1bd", 2 * H), ("ec_W2", H),
            ("gin_W1", 128), ("gin_W2", H), ("gate_W1", 128),
            ("gate_W2", 1), ("gcnb_bc", H), ("gatb_bc", H), ("ubvb", 2 * H)]
WPACK_F32 = [("fc_W", OUT), ("fc_b", OUT), ("ecb2_c", 1), ("ginb1_c", 1),
             ("ginb2_c", 1), ("gateb1_c", 1), ("iota64", G),
             ("dinv", NB), ("bids", NB), ("npad", NB)]


def _offsets(spec):
    out, o = {}, 0
    for n, w in spec:
        out[n] = (o, w)
        o += w
    return out, o


OFF_BF, W_BF = _offsets(WPACK_BF)
OFF_F32, W_F32 = _offsets(WPACK_F32)


# AllGather chunking (in blocks) so table AG overlaps the producing stage.
class _Layout:
    def __init__(self, chunks):
        self.chunks = chunks
        self.start = np.cumsum((0,) + chunks)[:-1]
        self.rows = np.array(chunks) * 128
        self.tab_base = np.cumsum([0] + [R * r for r in self.rows])[:-1]
        self.of_block = np.repeat(np.arange(len(chunks)), chunks)

    def row_of_gslot(self, gs):
        gs = np.asarray(gs)
        r, s = gs // NPCP, gs % NPCP
        b = s // 128
        c = self.of_block[b]
        return (self.tab_base[c] + r * self.rows[c]
                + (s - 128 * self.start[c])).astype(np.int32)


LA = _Layout((17, 16, 16))
LB = _Layout((11, 11, 11, 8, 8))
assert sum(LA.chunks) == NB and sum(LB.chunks) == NB


def _csr_tables(es, ed, slot_of, row_of_node, sent_row, dup_pad):
    """Build per-core padded-CSR gather tables for edges (es -> ed).

    Returns (K[b] block slot counts, off[b] col offsets, idx [R,128,S],
    cnt [R,NB,128] true per-slot counts).
    """
    sg = slot_of[ed]                       # global dst slot
    order = np.lexsort((row_of_node[es], sg))
    es_s, sg_s = es[order], sg[order]
    counts = np.bincount(sg_s, minlength=R * NPCP)
    starts = np.concatenate(([0], np.cumsum(counts)))[:-1]
    k_of = np.arange(len(sg_s)) - starts[sg_s]
    K = counts.reshape(R, NB, 128).max(axis=(0, 2))   # common across cores
    off = np.concatenate(([0], np.cumsum(K)))
    S = int(off[-1])
    idx = np.empty((R, 128, S), np.int32)
    idx[:] = sent_row[:, None, None]
    r_e, s_e = sg_s // NPCP, sg_s % NPCP
    b_e, p_e = s_e // 128, s_e % 128
    idx[r_e, p_e, off[b_e] + k_of] = row_of_node[es_s]
    cnt = counts.reshape(R, NB, 128).copy()
    if dup_pad:
        # replace sentinel padding with a copy of the last real edge (exact
        # for segment-max); slots with zero edges keep the sentinel.
        for b in range(NB):
            kb = int(K[b])
            if kb == 0:
                continue
            cols = np.arange(off[b], off[b] + kb)
            lastc = off[b] + np.maximum(cnt[:, b, :] - 1, 0)   # [R,128]
            last = np.take_along_axis(
                idx, lastc[:, :, None], axis=2)                # [R,128,1]
            have = (cnt[:, b, :, None] > np.arange(kb))
            nonzero = cnt[:, b, :, None] > 0
            blk = idx[:, :, cols]
            idx[:, :, cols] = np.where(have, blk, np.where(nonzero, last, blk))
    return K, off, idx, cnt


def _make_groups(K, lay, tgt):
    """Greedy-pack consecutive blocks into chunk-aligned gather groups."""
    groups = []
    for c, nb in enumerate(lay.chunks):
        b0 = int(lay.start[c])
        b = b0
        while b < b0 + nb:
            e, s = b, 0
            while e < b0 + nb and (e == b or s + K[e] <= tgt):
                s += K[e]
                e += 1
            groups.append((b, e, c))
            b = e
    return groups


def _preprocess(x, edge_index, batch, gcn_W, gcn_b, gat_W, att_src, att_dst,
                gat_b, ec_W1, ec_b1, ec_W2, ec_b2, gin_W1, gin_b1, gin_W2,
                gin_b2, gate_W1, gate_b1, gate_W2, gate_b2, fc_W, fc_b):
    src = np.asarray(edge_index[0], np.int64)
    dst = np.asarray(edge_index[1], np.int64)
    x = np.asarray(x, np.float32)
    batch = np.asarray(batch, np.int64)

    deg2 = np.bincount(dst, minlength=N)            # in-degree w/o self-loop
    dinv = (1.0 / np.sqrt((deg2 + 1).astype(np.float64))).astype(np.float32)

    # per-core permutation: sort own nodes by in-degree descending
    perm = np.empty((R, NPC), np.int64)
    for r in range(R):
        base = r * NPC
        perm[r] = base + np.argsort(-deg2[base:base + NPC], kind="stable")
    slot_of = np.empty(N, np.int64)                 # node -> global slot
    for r in range(R):
        slot_of[perm[r]] = r * NPCP + np.arange(NPC)
    rowA_of_node = LA.row_of_gslot(slot_of)         # node -> z/g table row
    rowB_of_node = LB.row_of_gslot(slot_of)         # node -> u/h3 table row
    sentA = LA.row_of_gslot(np.arange(R) * NPCP + (NPCP - 1))
    sentB = LB.row_of_gslot(np.arange(R) * NPCP + (NPCP - 1))

    loops = np.arange(N)
    es1 = np.concatenate([src, loops])
    ed1 = np.concatenate([dst, loops])
    K1, off1, idx1, cnt1 = _csr_tables(es1, ed1, slot_of, rowA_of_node,
                                       sentA, dup_pad=False)
    K2, off2, idx2ec, _ = _csr_tables(src, dst, slot_of, rowB_of_node,
                                      sentB, dup_pad=True)
    _, _, idx2gin, _ = _csr_tables(src, dst, slot_of, rowB_of_node,
                                   sentB, dup_pad=False)

    # replicated z-table: z = (dinv * x) @ gcn_W (zeros on padding rows), fp8
    gcn_W = np.asarray(gcn_W, np.float32)
    z_full = (x * dinv[:, None]) @ gcn_W
    assert np.abs(z_full).max() < 15.0, "z overflows fp8-e3m4 range"
    z_tab = np.zeros((TABR, H), np_fp8)
    z_tab[rowA_of_node] = z_full.astype(np_fp8)

    # derived weights (host)
    gat_W = np.asarray(gat_W, np.float32)
    att_src = np.asarray(att_src, np.float32)
    att_dst = np.asarray(att_dst, np.float32)
    B_src = np.einsum("fhc,hc->fh",
                      gat_W.reshape(IN, HEADS, C), att_src).astype(np.float32)
    B_dst = np.einsum("fhc,hc->fh",
                      gat_W.reshape(IN, HEADS, C), att_dst).astype(np.float32)
    gat_cat = np.concatenate([gat_W, B_src, B_dst], axis=1)   # [128,136]
    ec_W1 = np.asarray(ec_W1, np.float32)
    W1a, W1b = ec_W1[:H], ec_W1[H:]
    W1d = (W1a - W1b).astype(np.float32)
    W1bd = np.concatenate([W1b, W1d], axis=1)                 # [128,256]
    ubvb = np.concatenate([(-W1b.sum(0)),
                           np.asarray(ec_b1, np.float32) - W1d.sum(0)])

    wbf_c = np.zeros((128, W_BF), np.float32)

    def put_bf(n, a):
        o, w = OFF_BF[n]
        a = np.asarray(a, np.float32)
        wbf_c[:a.shape[0], o:o + w] = a.reshape(a.shape[0], w)

    put_bf("gat_cat", gat_cat)
    put_bf("W1bd", W1bd)
    put_bf("ec_W2", np.asarray(ec_W2, np.float32))
    put_bf("gin_W1", np.asarray(gin_W1, np.float32))
    put_bf("gin_W2", np.asarray(gin_W2, np.float32))
    put_bf("gate_W1", np.asarray(gate_W1, np.float32))
    put_bf("gate_W2", np.asarray(gate_W2, np.float32).reshape(H, 1))
    put_bf("gcnb_bc", np.tile(np.asarray(gcn_b, np.float32), (128, 1)))
    put_bf("gatb_bc", np.tile(np.asarray(gat_b, np.float32), (128, 1)))
    put_bf("ubvb", ubvb.reshape(1, 2 * H))

    wf_const = np.zeros((128, W_F32), np.float32)

    def put_f32(arr, n, a):
        o, w = OFF_F32[n]
        a = np.asarray(a, np.float32)
        arr[:a.shape[0], o:o + w] = a.reshape(a.shape[0], w)

    put_f32(wf_const, "fc_W", np.asarray(fc_W, np.float32))
    put_f32(wf_const, "fc_b", np.asarray(fc_b, np.float32).reshape(1, OUT))
    put_f32(wf_const, "ecb2_c", np.asarray(ec_b2, np.float32).reshape(H, 1))
    put_f32(wf_const, "ginb1_c", np.asarray(gin_b1, np.float32).reshape(128, 1))
    put_f32(wf_const, "ginb2_c", np.asarray(gin_b2, np.float32).reshape(H, 1))
    put_f32(wf_const, "gateb1_c",
            np.asarray(gate_b1, np.float32).reshape(128, 1))
    put_f32(wf_const, "iota64",
            np.tile(np.arange(G, dtype=np.float32), (128, 1)))

    per_core = []
    for r in range(R):
        wf_c = wf_const.copy()
        dv = np.zeros((NB * 128,), np.float32)
        dv[:NPC] = dinv[perm[r]]
        put_f32(wf_c, "dinv", dv.reshape(NB, 128).T)
        bd = np.full((NB * 128,), 999.0, np.float32)
        bd[:NPC] = batch[perm[r]].astype(np.float32)
        put_f32(wf_c, "bids", bd.reshape(NB, 128).T)
        npad = (K1[None, :, None] - cnt1[r]).astype(np.float32)  # [NB,128]
        flat = npad.reshape(-1)
        flat[NPC:] = 0.0     # dummy slots: numerator is exactly 0
        put_f32(wf_c, "npad", npad.reshape(NB, 128).T)
        per_core.append({
            "z_tab": z_tab,
            "wpack_bf": wbf_c.astype(np_bf16),
            "wpack_f32": wf_c,
            "idx1": np.ascontiguousarray(idx1[r]),
            "idx2ec": np.ascontiguousarray(idx2ec[r]),
            "idx2gin": np.ascontiguousarray(idx2gin[r]),
        })
    meta = {
        "K1": [int(k) for k in K1], "off1": [int(o) for o in off1],
        "K2": [int(k) for k in K2], "off2": [int(o) for o in off2],
        "S1": int(off1[-1]), "S2": int(off2[-1]),
        "gate_b2": float(np.asarray(gate_b2).reshape(-1)[0]),
        "perm": perm,
    }
    return per_core, meta


class _Ledger:
    """Per-stage DVE/Pool load ledger using the TRN2 cost model rates."""

    def __init__(self):
        self.v = 0.0
        self.p = 0.0

    @staticmethod
    def dve_cost(elems, fast):
        return elems * (0.52 if fast else 1.04) + 170.0

    @staticmethod
    def pool_cost(elems):
        return elems * (0.834 / 0.42) + 260.0

    def pick(self, elems, fast=False, nops=1):
        cv = elems * (0.52 if fast else 1.04) + 170.0 * nops
        cp = elems * (0.834 / 0.42) + 260.0 * nops
        if self.v + cv <= self.p + cp:
            self.v += cv
            return "v"
        self.p += cp
        return "p"

    def charge_pool(self, ns):
        self.p += ns

    def charge_dve(self, ns):
        self.v += ns


def _fold_sum(nc, src3, fold_t, K, D, eng1):
    """Sum K slots of src3 [128,K,D] into bf16 fold_t [128,ceil(K/2)*D].
    Level 1 reads the source dtype on eng1; the bf16 tree stays on DVE.
    Returns AP [128, D]."""
    f3 = fold_t[:].rearrange("p (k d) -> p k d", d=D)
    if K == 1:
        eng1.tensor_copy(out=f3[:, 0, :], in_=src3[:, 0, :])
        return fold_t[:, :D]
    h = K // 2
    eng1.tensor_tensor(out=f3[:, :h, :], in0=src3[:, :h, :],
                       in1=src3[:, K - h:K, :], op=OP.add)
    if K & 1:
        eng1.tensor_copy(out=f3[:, h, :], in_=src3[:, h, :])
    k = K - h
    while k > 1:
        hh = k // 2
        nc.vector.tensor_tensor(out=f3[:, :hh, :], in0=f3[:, :hh, :],
                                in1=f3[:, k - hh:k, :], op=OP.add)
        k -= hh
    return fold_t[:, :D]


def _tree(nc, f3, k, eng):
    """In-place bf16 tree-sum of k slots of f3 [128, k, D] into slot 0."""
    while k > 1:
        hh = k // 2
        eng.tensor_tensor(out=f3[:, :hh], in0=f3[:, :hh],
                          in1=f3[:, k - hh:k], op=OP.add)
        k -= hh


def _build(meta):
    K1, off1, S1 = meta["K1"], meta["off1"], meta["S1"]
    K2, off2, S2 = meta["K2"], meta["off2"], meta["S2"]
    gate_b2 = meta["gate_b2"]
    groups1A = _make_groups(K1, LA, GTGT1)  # stage 1: stages g shards (A)
    groups1B = _make_groups(K1, LB, GTGT)   # stage 2: stages u shards (B)
    groups2B = _make_groups(K2, LB, GTGT)   # stages 3/4: h3 shards (B)

    nc = bacc.Bacc("TRN2", target_bir_lowering=False, debug=False,
                   num_devices=R)

    def din(name, shape, dt=f32):
        return nc.dram_tensor(name, shape, dt, kind="ExternalInput")

    zP = din("z_tab", [TABR, H], fp8)
    idx1P = din("idx1", [128, S1], i32)
    idx2ecP = din("idx2ec", [128, S2], i32)
    idx2ginP = din("idx2gin", [128, S2], i32)
    wbfP = din("wpack_bf", [128, W_BF], bf16)
    wf32P = din("wpack_f32", [128, W_F32], f32)
    outP = nc.dram_tensor("out", [G, OUT], f32, kind="ExternalOutput")

    # internal DRAM: per-chunk local shards + replicated Shared tables
    def shards(name, w, lay):
        return [nc.dram_tensor(f"{name}_c{c}", [int(lay.rows[c]), w], fp8)
                for c in range(len(lay.chunks))]
    g_sh = shards("g_sh", GW, LA)
    u_sh, h3_sh = shards("u_sh", H, LB), shards("h3_sh", H, LB)
    g_tab = nc.dram_tensor("g_tab", [TABR, GW], fp8, addr_space="Shared")
    u_tab = nc.dram_tensor("u_tab", [TABR, H], fp8, addr_space="Shared")
    h3_tab = nc.dram_tensor("h3_tab", [TABR, H], fp8, addr_space="Shared")
    ar_in = nc.dram_tensor("ar_in", [G, 132], f32)
    ar_out = nc.dram_tensor("ar_out", [R * G, 132], f32, addr_space="Shared")

    def ag(sh_list, tab, c, lay):
        base = int(lay.tab_base[c])
        rows = R * int(lay.rows[c])
        nc.gpsimd.collective_compute(
            "AllGather", OP.bypass, ins=[sh_list[c][:, :].opt()],
            outs=[tab[base:base + rows, :].opt()], replica_groups=REPL)

    with tile.TileContext(nc) as tc:
        with tc.tile_pool(name="cst", bufs=1) as cst, \
             tc.tile_pool(name="wrk", bufs=4) as wrk, \
             tc.tile_pool(name="gth", bufs=3) as gth, \
             tc.tile_pool(name="stg", bufs=2) as stg, \
             tc.tile_pool(name="psmm", bufs=2, space="PSUM") as psmm, \
             tc.tile_pool(name="pstr", bufs=2, space="PSUM") as pstr, \
             tc.tile_pool(name="ptb", bufs=2, space="PSUM") as ptbp, \
             tc.tile_pool(name="psa", bufs=1, space="PSUM") as psa, \
             tc.tile_pool(name="psacc", bufs=1, space="PSUM") as psacc:

            # index tables first: the first z gather depends only on idx1
            idx1_t = cst.tile([128, S1], i32)
            nc.sync.dma_start(out=idx1_t[:], in_=idx1P[:, :])
            idx2e_t = cst.tile([128, S2], i32)
            nc.sync.dma_start(out=idx2e_t[:], in_=idx2ecP[:, :])
            idx2g_t = cst.tile([128, S2], i32)
            nc.sync.dma_start(out=idx2g_t[:], in_=idx2ginP[:, :])
            wb_t = cst.tile([128, W_BF], bf16)
            nc.sync.dma_start(out=wb_t[:], in_=wbfP[:, :])
            wf_t = cst.tile([128, W_F32], f32)
            nc.sync.dma_start(out=wf_t[:], in_=wf32P[:, :])

            def WB(n):
                o, w = OFF_BF[n]
                return wb_t[:, o:o + w]

            def WF(n):
                o, w = OFF_F32[n]
                return wf_t[:, o:o + w]

            dinv_t, bids_t, npad_t = WF("dinv"), WF("bids"), WF("npad")

            ident = cst.tile([128, 128], f32)
            make_identity(nc, ident[:])
            identb = cst.tile([128, 128], bf16)
            nc.vector.tensor_copy(out=identb[:], in_=ident[:])
            adst_all = cst.tile([128, 4 * NB], bf16)
            cor_all = cst.tile([128, 4 * NB], f32)
            v_all = cst.tile([128, NB * H], bf16)
            h3self = cst.tile([128, NB * H], bf16)
            ones_t = cst.tile([128, 1], f32)
            nc.vector.memset(ones_t[:], 1.0)
            ones_row = cst.tile([1, G], f32)
            nc.vector.memset(ones_row[:], 1.0)
            onesb_row = cst.tile([1, 128], bf16)
            nc.vector.memset(onesb_row[:], 1.0)
            sentg = cst.tile([1, GW], fp8)
            nc.vector.memset(sentg[:, :H], 0.0)
            nc.vector.memset(sentg[:, H:], ASENT)
            sentu = cst.tile([1, H], fp8)
            nc.vector.memset(sentu[:], ASENT)
            zrow8 = cst.tile([1, H], fp8)
            nc.vector.memset(zrow8[:], 0.0)
            gb2_t = cst.tile([1, 1], f32)
            nc.vector.memset(gb2_t[:], gate_b2)

            def transpose_bf(src_ap, name):
                pt = pstr.tile([128, 128], bf16, tag="tr", name=f"pt_{name}")
                nc.tensor.transpose(out=pt[:], in_=src_ap, identity=identb[:])
                st = wrk.tile([128, 128], bf16, tag=f"tr_{name}",
                              name=f"tr_{name}")
                nc.scalar.activation(st[:], pt[:], AF.Copy)
                return st

            def shard_rows(sh_list, b, lay):
                c = int(lay.of_block[b])
                return sh_list[c], (b - int(lay.start[c])) * 128

            def gat_gen_ns(slots):
                return 994.0 + 0.34 * 128 * slots + 600.0

            # ---------- stage 1: GCN aggregate (z-table) + GAT prep ---------
            led = _Ledger()
            for (b0, b1, c) in groups1A:
                nbg = b1 - b0
                Sg = off1[b1] - off1[b0]
                zt = gth.tile([128, Sg * H], fp8, tag="gath", name="zt")
                nc.gpsimd.indirect_dma_start(
                    out=zt[:], out_offset=None, in_=zP[:, :],
                    in_offset=bass.IndirectOffsetOnAxis(
                        ap=idx1_t[:, off1[b0]:off1[b1]], axis=0))
                led.charge_pool(gat_gen_ns(Sg))
                gstg = stg.tile([128, nbg * GW], fp8, tag="gstg", name="gstg")
                for b in range(b0, b1):
                    K = K1[b]
                    base = off1[b] - off1[b0]
                    z3 = zt[:, base * H:(base + K) * H].rearrange(
                        "p (k d) -> p k d", k=K)
                    fold = wrk.tile([128, ((K + 1) // 2) * H], bf16,
                                    tag="fold", name="fold1")
                    e = led.pick((K // 2) * H, fast=False)
                    led.charge_dve((K - K // 2 - 1) * H * 0.52 + 340)
                    zagg = _fold_sum(nc, z3, fold, K, H,
                                     nc.vector if e == "v" else nc.gpsimd)
                    h1 = wrk.tile([128, H], bf16, name="h1")
                    nc.vector.scalar_tensor_tensor(
                        out=h1[:], in0=zagg, scalar=dinv_t[:, b:b + 1],
                        in1=WB("gcnb_bc"), op0=OP.mult, op1=OP.add)
                    led.charge_dve(H * 0.26 + 170)
                    nc.scalar.activation(h1[:], h1[:], AF.Relu)
                    h1T = transpose_bf(h1[:], "h1T")
                    pg = psmm.tile([128, IN + 2 * HEADS], f32, tag="mm",
                                    name="pg")
                    nc.tensor.matmul(out=pg[:], lhsT=h1T[:],
                                     rhs=WB("gat_cat"), start=True, stop=True)
                    j = b - b0
                    nc.scalar.activation(gstg[:, j * GW:(j + 1) * GW],
                                         pg[:, :GW], AF.Copy)
                    nc.vector.tensor_copy(out=adst_all[:, 4 * b:4 * b + 4],
                                          in_=pg[:, GW:GW + 4])
                sh, rb = shard_rows(g_sh, b0, LA)
                nc.sync.dma_start(
                    out=sh[rb:rb + nbg * 128, :].rearrange(
                        "(j p) w -> p j w", p=128),
                    in_=gstg[:].rearrange("p (j w) -> p j w", j=nbg))
                if b1 == NB:  # sentinel precedes the last chunk's AG
                    shS, rbS = shard_rows(g_sh, NB - 1, LA)
                    nc.sync.dma_start(out=shS[rbS + 127:rbS + 128, :],
                                      in_=sentg[:1, :])
                if b1 == NB or int(LA.of_block[b1]) != c:
                    ag(g_sh, g_tab, c, LA)

            # ------------- stage 2: GAT aggregate + u/v prep ---------------
            # hoisted softmax padding correction for all blocks:
            # cor = npad * exp(0.2*a_dst - 3.1)
            nc.vector.tensor_scalar(
                out=cor_all[:], in0=adst_all[:], scalar1=0.2, scalar2=-3.1,
                op0=OP.mult, op1=OP.add)
            nc.scalar.activation(cor_all[:], cor_all[:], AF.Exp)
            nc.vector.tensor_tensor(
                out=cor_all[:].rearrange("p (b h) -> p b h", h=4),
                in0=cor_all[:].rearrange("p (b h) -> p b h", h=4),
                in1=npad_t[:, :, None].to_broadcast([128, NB, 4]), op=OP.mult)

            led = _Ledger()
            for (b0, b1, c) in groups1B:
                nbg = b1 - b0
                Sg = off1[b1] - off1[b0]
                gt = gth.tile([128, Sg * GW], fp8, tag="gath", name="gt")
                nc.gpsimd.indirect_dma_start(
                    out=gt[:], out_offset=None, in_=g_tab[:, :],
                    in_offset=bass.IndirectOffsetOnAxis(
                        ap=idx1_t[:, off1[b0]:off1[b1]], axis=0))
                led.charge_pool(gat_gen_ns(Sg))
                ustg = stg.tile([128, nbg * H], fp8, tag="ustg", name="ustg")
                for b in range(b0, b1):
                    K = K1[b]
                    base = off1[b] - off1[b0]
                    g3 = gt[:, base * GW:(base + K) * GW].rearrange(
                        "p (k w) -> p k w", k=K)
                    # attention logits e = lrelu(a_src + a_dst), exp
                    et = wrk.tile([128, K * HEADS], bf16, tag="et", name="et")
                    e3 = et[:].rearrange("p (k h) -> p k h", k=K)
                    nc.vector.tensor_tensor(
                        out=e3, in0=g3[:, :, H:],
                        in1=adst_all[:, 4 * b:4 * b + 4][:, None, :]
                        .to_broadcast([128, K, HEADS]), op=OP.add)
                    nc.vector.scalar_tensor_tensor(
                        out=et[:], in0=et[:], scalar=0.2, in1=et[:],
                        op0=OP.mult, op1=OP.max)
                    led.charge_dve(K * HEADS * 2.1 + 340)
                    nc.scalar.activation(et[:], et[:], AF.Exp)
                    # weight g rows by exp(e) per head, then fold sums
                    wtf = wrk.tile([128, K * H], bf16, tag="kbuf", name="wtf")
                    w3 = wtf[:].rearrange("p (k d) -> p k d", k=K)
                    g4 = g3[:, :, :H].rearrange("p k (h c) -> p k h c",
                                                h=HEADS)
                    w4 = w3.rearrange("p k (h c) -> p k h c", h=HEADS)
                    e4 = e3[:, :, :, None].to_broadcast([128, K, HEADS, C])
                    e = led.pick(K * H, fast=False)
                    weng = nc.vector if e == "v" else nc.gpsimd
                    weng.tensor_tensor(out=w4, in0=g4, in1=e4, op=OP.mult)
                    _tree(nc, w3, K, nc.vector)
                    led.charge_dve((K - 1) * H * 0.52 + 850)
                    _tree(nc, e3, K, nc.vector)
                    led.charge_dve(K * HEADS * 0.52 + 500)
                    # denominator with hoisted padding correction
                    den = wrk.tile([128, HEADS], f32, name="den")
                    nc.vector.scalar_tensor_tensor(
                        out=den[:], in0=cor_all[:, 4 * b:4 * b + 4],
                        scalar=-1.0, in1=et[:, :HEADS],
                        op0=OP.mult, op1=OP.add)
                    rd = wrk.tile([128, HEADS], f32, name="rd")
                    nc.vector.reciprocal(rd[:], den[:])
                    h2 = wrk.tile([128, H], bf16, name="h2")
                    h2v = h2[:].rearrange("p (h c) -> p h c", h=HEADS)
                    nc.vector.tensor_tensor(
                        out=h2v,
                        in0=wtf[:, :H].rearrange("p (h c) -> p h c", h=HEADS),
                        in1=rd[:][:, :, None].to_broadcast([128, HEADS, C]),
                        op=OP.mult)
                    nc.vector.tensor_tensor(out=h2[:], in0=h2[:],
                                            in1=WB("gatb_bc"), op=OP.add)
                    led.charge_dve(2 * H * 1.04 + 500)
                    # elu + 1 (the -1 is folded into ubvb)
                    ng = wrk.tile([128, H], bf16, name="ng")
                    nc.vector.tensor_scalar_min(ng[:], h2[:], 0.0)
                    nc.scalar.activation(ng[:], ng[:], AF.Exp)
                    nc.vector.scalar_tensor_tensor(
                        out=h2[:], in0=h2[:], scalar=0.0, in1=ng[:],
                        op0=OP.max, op1=OP.add)
                    led.charge_dve(2 * H * 0.3 + 340)
                    h2T = transpose_bf(h2[:], "h2T")
                    pu = psmm.tile([128, 2 * H], f32, tag="mm", name="pu")
                    nc.tensor.matmul(out=pu[:], lhsT=h2T[:], rhs=WB("W1bd"),
                                     start=True, stop=False)
                    nc.tensor.matmul(out=pu[:], lhsT=onesb_row[:1, :],
                                     rhs=WB("ubvb")[:1, :], start=False,
                                     stop=True)
                    j = b - b0
                    nc.scalar.activation(ustg[:, j * H:(j + 1) * H],
                                         pu[:, :H], AF.Copy)
                    nc.scalar.activation(v_all[:, b * H:(b + 1) * H],
                                         pu[:, H:], AF.Copy)
                sh, rb = shard_rows(u_sh, b0, LB)
                nc.sync.dma_start(
                    out=sh[rb:rb + nbg * 128, :].rearrange(
                        "(j p) w -> p j w", p=128),
                    in_=ustg[:].rearrange("p (j w) -> p j w", j=nbg))
                if b1 == NB:
                    shS, rbS = shard_rows(u_sh, NB - 1, LB)
                    nc.sync.dma_start(out=shS[rbS + 127:rbS + 128, :],
                                      in_=sentu[:1, :])
                if b1 == NB or int(LB.of_block[b1]) != c:
                    ag(u_sh, u_tab, c, LB)

            # ---------------- stage 3: EdgeConv ----------------------------
            led = _Ledger()
            for (b0, b1, c) in groups2B:
                nbg = b1 - b0
                Sg = off2[b1] - off2[b0]
                ut = gth.tile([128, max(Sg, 1) * H], fp8, tag="gath",
                              name="ut")
                if Sg > 0:
                    nc.gpsimd.indirect_dma_start(
                        out=ut[:, :Sg * H], out_offset=None, in_=u_tab[:, :],
                        in_offset=bass.IndirectOffsetOnAxis(
                            ap=idx2e_t[:, off2[b0]:off2[b1]], axis=0))
                    led.charge_pool(gat_gen_ns(Sg))
                hstg = stg.tile([128, nbg * H], fp8, tag="hstg", name="hstg")
                for b in range(b0, b1):
                    K = K2[b]
                    base = off2[b] - off2[b0]
                    assert K > 0
                    u3 = ut[:, base * H:(base + K) * H].rearrange(
                        "p (k d) -> p k d", k=K)
                    uf = wrk.tile([128, K * H], bf16, tag="kbuf", name="uf")
                    u3f = uf[:].rearrange("p (k d) -> p k d", k=K)
                    e = led.pick(K * H, fast=False)
                    aeng = nc.vector if e == "v" else nc.gpsimd
                    aeng.tensor_tensor(
                        out=u3f, in0=u3,
                        in1=v_all[:, b * H:(b + 1) * H][:, None, :]
                        .to_broadcast([128, K, H]), op=OP.add)
                    G4 = (K + 3) // 4
                    red = wrk.tile([128, G4 * 128], bf16, tag="red",
                                   name="red")
                    k0 = 0
                    gi = 0
                    while k0 < K:
                        q = min(4, K - k0)
                        pt = ptbp.tile([128, q * 128], bf16, tag="ptb",
                                       name="ec_pt")
                        for j in range(q):
                            nc.tensor.transpose(
                                out=pt[:, j * 128:(j + 1) * 128],
                                in_=u3f[:, k0 + j, :], identity=identb[:])
                        m1 = wrk.tile([128, q * 128], bf16, tag="m1",
                                      name="m1")
                        nc.scalar.activation(m1[:], pt[:], AF.Relu)
                        pm = pstr.tile([128, q * 128], f32, tag="tr",
                                        name="ec_pm")
                        nc.tensor.matmul(out=pm[:], lhsT=WB("ec_W2"),
                                         rhs=m1[:], start=True, stop=True)
                        if q > 1:
                            nc.vector.tensor_reduce(
                                out=red[:, gi * 128:(gi + 1) * 128].rearrange(
                                    "p (n o) -> p n o", o=1),
                                in_=pm[:].rearrange(
                                    "p (j n) -> p n j", j=q),
                                axis=mybir.AxisListType.X, op=OP.max)
                            led.charge_dve(q * 128 * 1.04 + 300)
                        else:
                            nc.vector.tensor_copy(
                                out=red[:, gi * 128:(gi + 1) * 128],
                                in_=pm[:, :128])
                            led.charge_dve(128 * 1.04 + 300)
                        k0 += q
                        gi += 1
                    h3T = wrk.tile([128, H], bf16, name="h3T")
                    if G4 > 1:
                        accT = wrk.tile([128, H], bf16, name="accT")
                        nc.vector.tensor_reduce(
                            out=accT[:].rearrange("p (n o) -> p n o", o=1),
                            in_=red[:].rearrange("p (g n) -> p n g", g=G4),
                            axis=mybir.AxisListType.X, op=OP.max)
                        led.charge_dve(G4 * 128 * 1.04 + 170)
                        nc.scalar.activation(h3T[:], accT[:], AF.Relu,
                                             bias=WF("ecb2_c")[:, :1])
                    else:
                        nc.scalar.activation(h3T[:], red[:, :128], AF.Relu,
                                             bias=WF("ecb2_c")[:, :1])
                    ph3 = psmm.tile([128, 128], bf16, tag="mm", name="ph3")
                    nc.tensor.transpose(out=ph3[:], in_=h3T[:],
                                        identity=identb[:])
                    j = b - b0
                    nc.scalar.activation(hstg[:, j * H:(j + 1) * H], ph3[:],
                                         AF.Copy)
                    nc.vector.tensor_copy(
                        out=h3self[:, b * H:(b + 1) * H], in_=ph3[:])
                    led.charge_dve(H * 1.04 + 170)
                sh, rb = shard_rows(h3_sh, b0, LB)
                nc.sync.dma_start(
                    out=sh[rb:rb + nbg * 128, :].rearrange(
                        "(j p) w -> p j w", p=128),
                    in_=hstg[:].rearrange("p (j w) -> p j w", j=nbg))
                if b1 == NB:
                    shS, rbS = shard_rows(h3_sh, NB - 1, LB)
                    nc.sync.dma_start(out=shS[rbS + 127:rbS + 128, :],
                                      in_=zrow8[:1, :])
                if b1 == NB or int(LB.of_block[b1]) != c:
                    ag(h3_sh, h3_tab, c, LB)

            # ---------------- stage 4: GIN + gated pooling -----------------
            led = _Ledger()
            ppd = psacc.tile([G, H + 1], f32, name="ppd")
            pp = ppd[:, :H]
            pd = ppd[:, H:H + 1]
            bi = 0  # running block index for the accumulate chain
            for (b0, b1, c) in groups2B:
                Sg = off2[b1] - off2[b0]
                st = gth.tile([128, max(Sg, 1) * H], fp8, tag="gath",
                              name="st")
                if Sg > 0:
                    nc.gpsimd.indirect_dma_start(
                        out=st[:, :Sg * H], out_offset=None, in_=h3_tab[:, :],
                        in_offset=bass.IndirectOffsetOnAxis(
                            ap=idx2g_t[:, off2[b0]:off2[b1]], axis=0))
                    led.charge_pool(gat_gen_ns(Sg))
                for b in range(b0, b1):
                    K = K2[b]
                    base = off2[b] - off2[b0]
                    s = wrk.tile([128, H], bf16, name="s")
                    if K > 0:
                        s3 = st[:, base * H:(base + K) * H].rearrange(
                            "p (k d) -> p k d", k=K)
                        fold = wrk.tile([128, ((K + 1) // 2) * H], bf16,
                                        tag="fold", name="fold4")
                        e = led.pick((K // 2) * H, fast=False)
                        led.charge_dve((K - K // 2 - 1) * H * 0.52 + 340)
                        ssum = _fold_sum(nc, s3, fold, K, H,
                                         nc.vector if e == "v" else nc.gpsimd)
                        nc.vector.tensor_tensor(
                            out=s[:], in0=ssum,
                            in1=h3self[:, b * H:(b + 1) * H], op=OP.add)
                        led.charge_dve(H * 0.52 + 170)
                    else:
                        nc.vector.tensor_copy(
                            out=s[:], in_=h3self[:, b * H:(b + 1) * H])
                    sT = transpose_bf(s[:], "sT")
                    p1 = psmm.tile([128, 128], f32, tag="mm", name="p1")
                    nc.tensor.matmul(out=p1[:], lhsT=WB("gin_W1"),
                                     rhs=sT[:], start=True, stop=True)
                    t1 = wrk.tile([128, 128], bf16, name="t1")
                    nc.scalar.activation(t1[:], p1[:], AF.Relu,
                                         bias=WF("ginb1_c")[:, :1])
                    p2 = psmm.tile([128, H], f32, tag="mm", name="p2")
                    nc.tensor.matmul(out=p2[:], lhsT=WB("gin_W2"),
                                     rhs=t1[:], start=True, stop=True)
                    h4T = wrk.tile([128, H], bf16, name="h4T")
                    nc.scalar.activation(h4T[:], p2[:], AF.Relu,
                                         bias=WF("ginb2_c")[:, :1])
                    pg1 = psmm.tile([128, 128], f32, tag="mm", name="pg1")
                    nc.tensor.matmul(out=pg1[:], lhsT=WB("gate_W1"),
                                     rhs=h4T[:], start=True, stop=True)
                    g1 = wrk.tile([128, 128], bf16, name="g1")
                    nc.scalar.activation(g1[:], pg1[:], AF.Relu,
                                         bias=WF("gateb1_c")[:, :1])
                    pg2 = psa.tile([1, 128], f32, tag="psA", name="pg2")
                    nc.tensor.matmul(out=pg2[:], lhsT=WB("gate_W2"),
                                     rhs=g1[:], start=True, stop=True)
                    egT = wrk.tile([1, 128], f32, name="egT")
                    nc.scalar.activation(egT[:], pg2[:], AF.Exp,
                                         bias=gb2_t[:1, :1])
                    ph4 = psmm.tile([128, 128], bf16, tag="mm", name="ph4")
                    nc.tensor.transpose(out=ph4[:], in_=h4T[:],
                                        identity=identb[:])
                    h4r = wrk.tile([128, H], f32, name="h4r")
                    nc.scalar.activation(h4r[:], ph4[:], AF.Copy)
                    pe = psa.tile([128, 1], f32, tag="psA", name="pe")
                    nc.tensor.transpose(out=pe[:], in_=egT[:1, :],
                                        identity=ident[:1, :1])
                    oh = wrk.tile([128, G], f32, name="oh")
                    nc.vector.scalar_tensor_tensor(
                        out=oh[:], in0=WF("iota64"),
                        scalar=bids_t[:, b:b + 1],
                        in1=pe[:, :1].to_broadcast([128, G]),
                        op0=OP.is_equal, op1=OP.mult)
                    led.charge_dve(G * 1.04 + 170)
                    nc.tensor.matmul(out=pp, lhsT=oh[:], rhs=h4r[:],
                                     start=(bi == 0), stop=(bi == NB - 1))
                    nc.tensor.matmul(out=pd, lhsT=oh[:], rhs=ones_t[:, :1],
                                     start=(bi == 0), stop=(bi == NB - 1))
                    bi += 1

            # pooled partials -> AllGather -> local reduce -> dense head
            pl = wrk.tile([G, 132], f32, name="pl")
            nc.vector.memset(pl[:], 0.0)
            nc.vector.tensor_copy(out=pl[:, :H + 1], in_=ppd[:])
            nc.sync.dma_start(out=ar_in[:, :], in_=pl[:])
            nc.gpsimd.collective_compute(
                "AllGather", OP.bypass, ins=[ar_in[:, :].opt()],
                outs=[ar_out[:, :].opt()], replica_groups=REPL)
            ar = wrk.tile([G, R * 132], f32, name="ar")
            nc.sync.dma_start(
                out=ar[:].rearrange("g (r w) -> g r w", r=R),
                in_=ar_out[:, :].rearrange("(r g) w -> g r w", r=R))
            ard = wrk.tile([G, H + 1], f32, name="ard")
            nc.vector.tensor_reduce(
                out=ard[:].rearrange("g (n o) -> g n o", o=1),
                in_=ar[:].rearrange("g (r w) -> g w r", r=R)[:, :H + 1, :],
                axis=mybir.AxisListType.X, op=OP.add)
            rdn = wrk.tile([G, 1], f32, name="rdn")
            nc.vector.reciprocal(rdn[:], ard[:, H:H + 1])
            pooled = wrk.tile([G, H], f32, name="pooled")
            nc.vector.tensor_scalar_mul(pooled[:], ard[:, :H], rdn[:, :1])
            ppT = psmm.tile([128, G], f32, tag="mm", name="ppT")
            nc.tensor.transpose(out=ppT[:], in_=pooled[:G, :],
                                identity=ident[:G, :G])
            plT = wrk.tile([128, G], f32, name="plT")
            nc.vector.tensor_copy(out=plT[:], in_=ppT[:])
            psl = psa.tile([G, OUT], f32, tag="psA", name="psl")
            nc.tensor.matmul(out=psl[:], lhsT=plT[:], rhs=WF("fc_W"),
                             start=True, stop=False)
            nc.tensor.matmul(out=psl[:], lhsT=ones_row[:1, :],
                             rhs=WF("fc_b")[:1, :], start=False, stop=True)
            rmx = wrk.tile([G, 1], f32, name="rmx")
            nc.vector.tensor_reduce(out=rmx[:], in_=psl[:],
                                    axis=mybir.AxisListType.X, op=OP.max)
            xs = wrk.tile([G, OUT], f32, name="xs")
            nc.vector.tensor_scalar(out=xs[:], in0=psl[:], scalar1=rmx[:, :1],
                                    scalar2=None, op0=OP.subtract)
            ex = wrk.tile([G, OUT], f32, name="ex")
            ssum = wrk.tile([G, 1], f32, name="ssum")
            nc.scalar.activation(ex[:], xs[:], AF.Exp, accum_out=ssum[:, :1])
            lg = wrk.tile([G, 1], f32, name="lg")
            nc.scalar.activation(lg[:], ssum[:], AF.Ln)
            fin = wrk.tile([G, OUT], f32, name="fin")
            nc.vector.tensor_scalar(out=fin[:], in0=xs[:], scalar1=lg[:, :1],
                                    scalar2=None, op0=OP.subtract)
            nc.sync.dma_start(out=outP[:, :], in_=fin[:])

    nc.compile()
    return nc


_CACHE = {}


def kernel(**inputs) -> np.ndarray:
    per_core, meta = _preprocess(**inputs)
    key = (tuple(meta["K1"]), tuple(meta["K2"]))
    if key not in _CACHE:
        _CACHE[key] = _build(meta)
    nc = _CACHE[key]
    res = run_bass_kernel_spmd(nc, per_core, list(range(R)))
    return np.asarray(res.results[0]["out"], np.float32)


if __name__ == "__main__":
    import reference
    inputs = {k: np.asarray(v) for k, v in reference.setup_inputs().items()}
    got = kernel(**inputs)
    print(got[:4])


# revision 38
# speedup vs baseline: 12.3802x; 12.3802x over previous
"""Trainium2 Bass kernel for nn_AdvancedGCN (GCN -> GAT -> EdgeConv -> GIN ->
global-attention pooling) over N=50000 nodes / E=800000 edges, SPMD on 8
NeuronCores.

Strategy (v4): nodes are sharded 6250/core (padded to 6272 = 49 blocks of 128)
and sorted by in-degree DESCENDING (an ascending sort starts the AllGather
chains earlier but tightens the stage-boundary slack below what the runtime's
collective completion-vs-data-arrival skew tolerates: intermittent NaNs).  All graph index work happens on host and is
baked into int32 gather-index tables; the device program is pure dense compute.

v4 highlights vs v2:
 - GCN stage gathers a host-precomputed replicated z = (dinv*x)@gcn_W table
   (fp8); stage 1 has no matmul/extra transpose per block,
 - stage-1's three matmuls (gat_W / B_src / B_dst) fuse into one 136-wide
   matmul; stage-2's W1b / W1d fuse into one 256-wide matmul; the pooling
   numerator/denominator fuse into one accumulation chain via a ones column,
 - all small weights ship in two packed tensors (2 DMAs); index tables load
   first so the first gather issues at ~2us,
 - AllGather chunk shapes (6,15,24,4)/(4,9,11,12,9,4) tuned so chains start
   early and end with a small tail; GAT pad-correction hoisted out of the
   block loop; EdgeConv max uses grouped reduces into one wide SBUF tile
   plus a single strided final reduce,
 - DVE/Pool work split per block by a cost-model ledger that also charges
   Pool for SWDGE descriptor generation of the indirect gathers,
 - final pooling combine stays AllReduce + DRAM readback (an AllGather tail
   showed a data-arrival race on hardware; AllReduce is the proven-safe
   pattern).
"""
import os
import sys

import numpy as np
import ml_dtypes

for _p in ("/opt/trn_rl_repo", "/root/.axon_site/_ro/trn_rl_repo"):
    if os.path.isdir(_p) and _p not in sys.path:
        sys.path.insert(0, _p)

import concourse.bass as bass
import concourse.bacc as bacc
import concourse.tile as tile
import concourse.mybir as mybir
from concourse.bass_utils import run_bass_kernel_spmd
from concourse.masks import make_identity

N, E, IN, H, G, OUT = 50000, 800000, 128, 128, 64, 10
HEADS, C = 4, 32
R = 8                    # cores
NPC = N // R             # 6250 nodes per core
NB = (NPC + 127) // 128  # 49 blocks per core
NPCP = NB * 128          # 6272 padded nodes per core
TABR = R * NPCP          # replicated table rows
GW = 132                 # g-table row width (128 g + 4 a_src)
ASENT = -15.5            # fp8-e3m4 min: sentinel a_src / u value
GTGT = 80                # target gather-group K-sum, fp8 tables
GTGT1 = 72               # target gather-group K-sum, z table
f32, i32 = mybir.dt.float32, mybir.dt.int32
bf16 = mybir.dt.bfloat16
fp8 = mybir.dt.float8e3
np_fp8 = ml_dtypes.float8_e3m4
np_bf16 = ml_dtypes.bfloat16
AF = mybir.ActivationFunctionType
OP = mybir.AluOpType
REPL = [list(range(R))]

# packed-weight layouts: (name, cols); bf16 pack and f32 pack
WPACK_BF = [("gat_cat", IN + 2 * HEADS), ("W1bd", 2 * H), ("ec_W2", H),
            ("gin_W1", 128), ("gin_W2", H), ("gate_W1", 128),
            ("gate_W2", 1), ("gcnb_bc", H), ("gatb_bc", H), ("ubvb", 2 * H)]
WPACK_F32 = [("fc_W", OUT), ("fc_b", OUT), ("ecb2_c", 1), ("ginb1_c", 1),
             ("ginb2_c", 1), ("gateb1_c", 1), ("iota64", G),
             ("dinv", NB), ("bids", NB), ("npad", NB)]


def _offsets(spec):
    out, o = {}, 0
    for n, w in spec:
        out[n] = (o, w)
        o += w
    return out, o


OFF_BF, W_BF = _offsets(WPACK_BF)
OFF_F32, W_F32 = _offsets(WPACK_F32)


# AllGather chunking (in blocks) so table AG overlaps the producing stage.
class _Layout:
    def __init__(self, chunks):
        self.chunks = chunks
        self.start = np.cumsum((0,) + chunks)[:-1]
        self.rows = np.array(chunks) * 128
        self.tab_base = np.cumsum([0] + [R * r for r in self.rows])[:-1]
        self.of_block = np.repeat(np.arange(len(chunks)), chunks)

    def row_of_gslot(self, gs):
        gs = np.asarray(gs)
        r, s = gs // NPCP, gs % NPCP
        b = s // 128
        c = self.of_block[b]
        return (self.tab_base[c] + r * self.rows[c]
                + (s - 128 * self.start[c])).astype(np.int32)


LA = _Layout((6, 15, 24, 4))
LB = _Layout((4, 9, 11, 12, 9, 4))
assert sum(LA.chunks) == NB and sum(LB.chunks) == NB


def _csr_tables(es, ed, slot_of, row_of_node, sent_row, dup_pad):
    """Build per-core padded-CSR gather tables for edges (es -> ed).

    Returns (K[b] block slot counts, off[b] col offsets, idx [R,128,S],
    cnt [R,NB,128] true per-slot counts).
    """
    sg = slot_of[ed]                       # global dst slot
    order = np.lexsort((row_of_node[es], sg))
    es_s, sg_s = es[order], sg[order]
    counts = np.bincount(sg_s, minlength=R * NPCP)
    starts = np.concatenate(([0], np.cumsum(counts)))[:-1]
    k_of = np.arange(len(sg_s)) - starts[sg_s]
    K = counts.reshape(R, NB, 128).max(axis=(0, 2))   # common across cores
    off = np.concatenate(([0], np.cumsum(K)))
    S = int(off[-1])
    idx = np.empty((R, 128, S), np.int32)
    idx[:] = sent_row[:, None, None]
    r_e, s_e = sg_s // NPCP, sg_s % NPCP
    b_e, p_e = s_e // 128, s_e % 128
    idx[r_e, p_e, off[b_e] + k_of] = row_of_node[es_s]
    cnt = counts.reshape(R, NB, 128).copy()
    if dup_pad:
        # replace sentinel padding with a copy of the last real edge (exact
        # for segment-max); slots with zero edges keep the sentinel.
        for b in range(NB):
            kb = int(K[b])
            if kb == 0:
                continue
            cols = np.arange(off[b], off[b] + kb)
            lastc = off[b] + np.maximum(cnt[:, b, :] - 1, 0)   # [R,128]
            last = np.take_along_axis(
                idx, lastc[:, :, None], axis=2)                # [R,128,1]
            have = (cnt[:, b, :, None] > np.arange(kb))
            nonzero = cnt[:, b, :, None] > 0
            blk = idx[:, :, cols]
            idx[:, :, cols] = np.where(have, blk, np.where(nonzero, last, blk))
    return K, off, idx, cnt


def _make_groups(K, lay, tgt):
    """Greedy-pack consecutive blocks into chunk-aligned gather groups."""
    groups = []
    for c, nb in enumerate(lay.chunks):
        b0 = int(lay.start[c])
        b = b0
        while b < b0 + nb:
            e, s = b, 0
            while (e < b0 + nb and (e == b or s + K[e] <= tgt)
                   and e - b < 12):
                s += K[e]
                e += 1
            groups.append((b, e, c))
            b = e
    return groups


def _preprocess(x, edge_index, batch, gcn_W, gcn_b, gat_W, att_src, att_dst,
                gat_b, ec_W1, ec_b1, ec_W2, ec_b2, gin_W1, gin_b1, gin_W2,
                gin_b2, gate_W1, gate_b1, gate_W2, gate_b2, fc_W, fc_b):
    src = np.asarray(edge_index[0], np.int64)
    dst = np.asarray(edge_index[1], np.int64)
    x = np.asarray(x, np.float32)
    batch = np.asarray(batch, np.int64)

    deg2 = np.bincount(dst, minlength=N)            # in-degree w/o self-loop
    dinv = (1.0 / np.sqrt((deg2 + 1).astype(np.float64))).astype(np.float32)

    # per-core permutation: sort own nodes by in-degree descending
    perm = np.empty((R, NPC), np.int64)
    for r in range(R):
        base = r * NPC
        perm[r] = base + np.argsort(-deg2[base:base + NPC], kind="stable")
    slot_of = np.empty(N, np.int64)                 # node -> global slot
    for r in range(R):
        slot_of[perm[r]] = r * NPCP + np.arange(NPC)
    rowA_of_node = LA.row_of_gslot(slot_of)         # node -> z/g table row
    rowB_of_node = LB.row_of_gslot(slot_of)         # node -> u/h3 table row
    sentA = LA.row_of_gslot(np.arange(R) * NPCP + (NPCP - 1))
    sentB = LB.row_of_gslot(np.arange(R) * NPCP + (NPCP - 1))

    loops = np.arange(N)
    es1 = np.concatenate([src, loops])
    ed1 = np.concatenate([dst, loops])
    K1, off1, idx1, cnt1 = _csr_tables(es1, ed1, slot_of, rowA_of_node,
                                       sentA, dup_pad=False)
    K2, off2, idx2ec, _ = _csr_tables(src, dst, slot_of, rowB_of_node,
                                      sentB, dup_pad=True)
    _, _, idx2gin, _ = _csr_tables(src, dst, slot_of, rowB_of_node,
                                   sentB, dup_pad=False)

    # replicated z-table: z = (dinv * x) @ gcn_W (zeros on padding rows), fp8
    gcn_W = np.asarray(gcn_W, np.float32)
    z_full = (x * dinv[:, None]) @ gcn_W
    assert np.abs(z_full).max() < 15.0, "z overflows fp8-e3m4 range"
    z_tab = np.zeros((TABR, H), np_fp8)
    z_tab[rowA_of_node] = z_full.astype(np_fp8)

    # derived weights (host)
    gat_W = np.asarray(gat_W, np.float32)
    att_src = np.asarray(att_src, np.float32)
    att_dst = np.asarray(att_dst, np.float32)
    B_src = np.einsum("fhc,hc->fh",
                      gat_W.reshape(IN, HEADS, C), att_src).astype(np.float32)
    B_dst = np.einsum("fhc,hc->fh",
                      gat_W.reshape(IN, HEADS, C), att_dst).astype(np.float32)
    gat_cat = np.concatenate([gat_W, B_src, B_dst], axis=1)   # [128,136]
    ec_W1 = np.asarray(ec_W1, np.float32)
    W1a, W1b = ec_W1[:H], ec_W1[H:]
    W1d = (W1a - W1b).astype(np.float32)
    W1bd = np.concatenate([W1b, W1d], axis=1)                 # [128,256]
    ubvb = np.concatenate([(-W1b.sum(0)),
                           np.asarray(ec_b1, np.float32) - W1d.sum(0)])

    wbf_c = np.zeros((128, W_BF), np.float32)

    def put_bf(n, a):
        o, w = OFF_BF[n]
        a = np.asarray(a, np.float32)
        wbf_c[:a.shape[0], o:o + w] = a.reshape(a.shape[0], w)

    put_bf("gat_cat", gat_cat)
    put_bf("W1bd", W1bd)
    put_bf("ec_W2", np.asarray(ec_W2, np.float32))
    put_bf("gin_W1", np.asarray(gin_W1, np.float32))
    put_bf("gin_W2", np.asarray(gin_W2, np.float32))
    put_bf("gate_W1", np.asarray(gate_W1, np.float32))
    put_bf("gate_W2", np.asarray(gate_W2, np.float32).reshape(H, 1))
    put_bf("gcnb_bc", np.tile(np.asarray(gcn_b, np.float32), (128, 1)))
    put_bf("gatb_bc", np.tile(np.asarray(gat_b, np.float32), (128, 1)))
    put_bf("ubvb", ubvb.reshape(1, 2 * H))

    wf_const = np.zeros((128, W_F32), np.float32)

    def put_f32(arr, n, a):
        o, w = OFF_F32[n]
        a = np.asarray(a, np.float32)
        arr[:a.shape[0], o:o + w] = a.reshape(a.shape[0], w)

    put_f32(wf_const, "fc_W", np.asarray(fc_W, np.float32))
    put_f32(wf_const, "fc_b", np.asarray(fc_b, np.float32).reshape(1, OUT))
    put_f32(wf_const, "ecb2_c", np.asarray(ec_b2, np.float32).reshape(H, 1))
    put_f32(wf_const, "ginb1_c", np.asarray(gin_b1, np.float32).reshape(128, 1))
    put_f32(wf_const, "ginb2_c", np.asarray(gin_b2, np.float32).reshape(H, 1))
    put_f32(wf_const, "gateb1_c",
            np.asarray(gate_b1, np.float32).reshape(128, 1))
    put_f32(wf_const, "iota64",
            np.tile(np.arange(G, dtype=np.float32), (128, 1)))

    per_core = []
    for r in range(R):
        wf_c = wf_const.copy()
        dv = np.zeros((NB * 128,), np.float32)
        dv[:NPC] = dinv[perm[r]]
        put_f32(wf_c, "dinv", dv.reshape(NB, 128).T)
        bd = np.full((NB * 128,), 999.0, np.float32)
        bd[:NPC] = batch[perm[r]].astype(np.float32)
        put_f32(wf_c, "bids", bd.reshape(NB, 128).T)
        npad = (K1[None, :, None] - cnt1[r]).astype(np.float32)  # [NB,128]
        flat = npad.reshape(-1)
        flat[NPC:] = 0.0     # dummy slots: numerator is exactly 0
        put_f32(wf_c, "npad", npad.reshape(NB, 128).T)
        per_core.append({
            "z_tab": z_tab,
            "wpack_bf": wbf_c.astype(np_bf16),
            "wpack_f32": wf_c,
            "idx1": np.ascontiguousarray(idx1[r]),
            "idx2ec": np.ascontiguousarray(idx2ec[r]),
            "idx2gin": np.ascontiguousarray(idx2gin[r]),
        })
    meta = {
        "K1": [int(k) for k in K1], "off1": [int(o) for o in off1],
        "K2": [int(k) for k in K2], "off2": [int(o) for o in off2],
        "S1": int(off1[-1]), "S2": int(off2[-1]),
        "gate_b2": float(np.asarray(gate_b2).reshape(-1)[0]),
        "perm": perm,
    }
    return per_core, meta


class _Ledger:
    """Per-stage DVE/Pool load ledger using the TRN2 cost model rates."""

    def __init__(self):
        self.v = 0.0
        self.p = 0.0

    @staticmethod
    def dve_cost(elems, fast):
        return elems * (0.52 if fast else 1.04) + 170.0

    @staticmethod
    def pool_cost(elems):
        return elems * (0.834 / 0.42) + 260.0

    def pick(self, elems, fast=False, nops=1):
        cv = elems * (0.52 if fast else 1.04) + 170.0 * nops
        cp = elems * 2.6 + 400.0 * nops
        if self.v + cv <= self.p + cp:
            self.v += cv
            return "v"
        self.p += cp
        return "p"

    def charge_pool(self, ns):
        self.p += ns

    def charge_dve(self, ns):
        self.v += ns


def _fold_sum(nc, src3, fold_t, K, D, eng1):
    """Sum K slots of src3 [128,K,D] into bf16 fold_t [128,ceil(K/2)*D].
    Level 1 reads the source dtype on eng1; the bf16 tree stays on DVE.
    Returns AP [128, D]."""
    f3 = fold_t[:].rearrange("p (k d) -> p k d", d=D)
    if K == 1:
        eng1.tensor_copy(out=f3[:, 0, :], in_=src3[:, 0, :])
        return fold_t[:, :D]
    h = K // 2
    eng1.tensor_tensor(out=f3[:, :h, :], in0=src3[:, :h, :],
                       in1=src3[:, K - h:K, :], op=OP.add)
    if K & 1:
        eng1.tensor_copy(out=f3[:, h, :], in_=src3[:, h, :])
    k = K - h
    while k > 1:
        hh = k // 2
        nc.vector.tensor_tensor(out=f3[:, :hh, :], in0=f3[:, :hh, :],
                                in1=f3[:, k - hh:k, :], op=OP.add)
        k -= hh
    return fold_t[:, :D]


def _tree(nc, f3, k, eng):
    """In-place bf16 tree-sum of k slots of f3 [128, k, D] into slot 0."""
    while k > 1:
        hh = k // 2
        eng.tensor_tensor(out=f3[:, :hh], in0=f3[:, :hh],
                          in1=f3[:, k - hh:k], op=OP.add)
        k -= hh


def _build(meta):
    K1, off1, S1 = meta["K1"], meta["off1"], meta["S1"]
    K2, off2, S2 = meta["K2"], meta["off2"], meta["S2"]
    gate_b2 = meta["gate_b2"]
    groups1A = _make_groups(K1, LA, GTGT1)  # stage 1: stages g shards (A)
    groups1B = _make_groups(K1, LB, GTGT)   # stage 2: stages u shards (B)
    groups2B = _make_groups(K2, LB, GTGT)   # stages 3/4: h3 shards (B)

    nc = bacc.Bacc("TRN2", target_bir_lowering=False, debug=False,
                   num_devices=R)

    def din(name, shape, dt=f32):
        return nc.dram_tensor(name, shape, dt, kind="ExternalInput")

    zP = din("z_tab", [TABR, H], fp8)
    idx1P = din("idx1", [128, S1], i32)
    idx2ecP = din("idx2ec", [128, S2], i32)
    idx2ginP = din("idx2gin", [128, S2], i32)
    wbfP = din("wpack_bf", [128, W_BF], bf16)
    wf32P = din("wpack_f32", [128, W_F32], f32)
    outP = nc.dram_tensor("out", [G, OUT], f32, kind="ExternalOutput")

    # internal DRAM: per-chunk local shards + replicated Shared tables
    def shards(name, w, lay):
        return [nc.dram_tensor(f"{name}_c{c}", [int(lay.rows[c]), w], fp8)
                for c in range(len(lay.chunks))]
    g_sh = shards("g_sh", GW, LA)
    u_sh, h3_sh = shards("u_sh", H, LB), shards("h3_sh", H, LB)
    g_tab = nc.dram_tensor("g_tab", [TABR, GW], fp8, addr_space="Shared")
    u_tab = nc.dram_tensor("u_tab", [TABR, H], fp8, addr_space="Shared")
    h3_tab = nc.dram_tensor("h3_tab", [TABR, H], fp8, addr_space="Shared")
    ar_in = nc.dram_tensor("ar_in", [G, 132], f32)
    ar_out = nc.dram_tensor("ar_out", [G, 132], f32, addr_space="Shared")

    def ag(sh_list, tab, c, lay):
        base = int(lay.tab_base[c])
        rows = R * int(lay.rows[c])
        nc.gpsimd.collective_compute(
            "AllGather", OP.bypass, ins=[sh_list[c][:, :].opt()],
            outs=[tab[base:base + rows, :].opt()], replica_groups=REPL)

    with tile.TileContext(nc) as tc:
        with tc.tile_pool(name="cst", bufs=1) as cst, \
             tc.tile_pool(name="wrk", bufs=4) as wrk, \
             tc.tile_pool(name="gth", bufs=3) as gth, \
             tc.tile_pool(name="stg", bufs=2) as stg, \
             tc.tile_pool(name="psmm", bufs=2, space="PSUM") as psmm, \
             tc.tile_pool(name="pstr", bufs=2, space="PSUM") as pstr, \
             tc.tile_pool(name="ptb", bufs=2, space="PSUM") as ptbp, \
             tc.tile_pool(name="psa", bufs=1, space="PSUM") as psa, \
             tc.tile_pool(name="psacc", bufs=1, space="PSUM") as psacc:

            # index tables first: the first z gather depends only on idx1
            idx1_t = cst.tile([128, S1], i32)
            nc.sync.dma_start(out=idx1_t[:], in_=idx1P[:, :])
            idx2e_t = cst.tile([128, S2], i32)
            nc.sync.dma_start(out=idx2e_t[:], in_=idx2ecP[:, :])
            idx2g_t = cst.tile([128, S2], i32)
            nc.sync.dma_start(out=idx2g_t[:], in_=idx2ginP[:, :])
            wb_t = cst.tile([128, W_BF], bf16)
            nc.sync.dma_start(out=wb_t[:], in_=wbfP[:, :])
            wf_t = cst.tile([128, W_F32], f32)
            nc.sync.dma_start(out=wf_t[:], in_=wf32P[:, :])

            def WB(n):
                o, w = OFF_BF[n]
                return wb_t[:, o:o + w]

            def WF(n):
                o, w = OFF_F32[n]
                return wf_t[:, o:o + w]

            dinv_t, bids_t, npad_t = WF("dinv"), WF("bids"), WF("npad")

            ident = cst.tile([128, 128], f32)
            make_identity(nc, ident[:])
            identb = cst.tile([128, 128], bf16)
            nc.vector.tensor_copy(out=identb[:], in_=ident[:])
            adst_all = cst.tile([128, 4 * NB], bf16)
            cor_all = cst.tile([128, 4 * NB], f32)
            v_all = cst.tile([128, NB * H], bf16)
            h3self = cst.tile([128, NB * H], bf16)
            ones_row = cst.tile([1, G], f32)
            nc.vector.memset(ones_row[:], 1.0)
            onesb_row = cst.tile([1, 128], bf16)
            nc.vector.memset(onesb_row[:], 1.0)
            sentg = cst.tile([1, GW], fp8)
            nc.vector.memset(sentg[:, :H], 0.0)
            nc.vector.memset(sentg[:, H:], ASENT)
            sentu = cst.tile([1, H], fp8)
            nc.vector.memset(sentu[:], ASENT)
            zrow8 = cst.tile([1, H], fp8)
            nc.vector.memset(zrow8[:], 0.0)
            gb2_t = cst.tile([1, 1], f32)
            nc.vector.memset(gb2_t[:], gate_b2)

            def transpose_bf(src_ap, name, func=AF.Copy):
                pt = pstr.tile([128, 128], bf16, tag="tr", name=f"pt_{name}")
                nc.tensor.transpose(out=pt[:], in_=src_ap, identity=identb[:])
                st = wrk.tile([128, 128], bf16, tag=f"tr_{name}",
                              name=f"tr_{name}")
                nc.scalar.activation(st[:], pt[:], func)
                return st

            def shard_rows(sh_list, b, lay):
                c = int(lay.of_block[b])
                return sh_list[c], (b - int(lay.start[c])) * 128

            def gat_gen_ns(slots):
                return 994.0 + 0.34 * 128 * slots + 600.0

            # ---------- stage 1: GCN aggregate (z-table) + GAT prep ---------
            led = _Ledger()
            for (b0, b1, c) in groups1A:
                nbg = b1 - b0
                Sg = off1[b1] - off1[b0]
                zt = gth.tile([128, Sg * H], fp8, tag="gath", name="zt")
                nc.gpsimd.indirect_dma_start(
                    out=zt[:], out_offset=None, in_=zP[:, :],
                    in_offset=bass.IndirectOffsetOnAxis(
                        ap=idx1_t[:, off1[b0]:off1[b1]], axis=0))
                led.charge_pool(gat_gen_ns(Sg))
                gstg = stg.tile([128, nbg * GW], fp8, tag="gstg", name="gstg")
                for b in range(b0, b1):
                    K = K1[b]
                    base = off1[b] - off1[b0]
                    z3 = zt[:, base * H:(base + K) * H].rearrange(
                        "p (k d) -> p k d", k=K)
                    fold = wrk.tile([128, ((K + 1) // 2) * H], bf16,
                                    tag="fold", name="fold1")
                    e = led.pick((K // 2) * H, fast=False)
                    led.charge_dve((K - K // 2 - 1) * H * 0.52 + 340)
                    zagg = _fold_sum(nc, z3, fold, K, H,
                                     nc.vector if e == "v" else nc.gpsimd)
                    h1 = wrk.tile([128, H], bf16, name="h1")
                    nc.vector.scalar_tensor_tensor(
                        out=h1[:], in0=zagg, scalar=dinv_t[:, b:b + 1],
                        in1=WB("gcnb_bc"), op0=OP.mult, op1=OP.add)
                    led.charge_dve(H * 0.26 + 170)
                    h1T = transpose_bf(h1[:], "h1T", func=AF.Relu)
                    pg = psmm.tile([128, IN + 2 * HEADS], f32, tag="mm",
                                    name="pg")
                    nc.tensor.matmul(out=pg[:], lhsT=h1T[:],
                                     rhs=WB("gat_cat"), start=True, stop=True)
                    j = b - b0
                    nc.scalar.activation(gstg[:, j * GW:(j + 1) * GW],
                                         pg[:, :GW], AF.Copy)
                    nc.vector.tensor_copy(out=adst_all[:, 4 * b:4 * b + 4],
                                          in_=pg[:, GW:GW + 4])
                sh, rb = shard_rows(g_sh, b0, LA)
                nc.sync.dma_start(
                    out=sh[rb:rb + nbg * 128, :].rearrange(
                        "(j p) w -> p j w", p=128),
                    in_=gstg[:].rearrange("p (j w) -> p j w", j=nbg))
                if b1 == NB:  # sentinel precedes the last chunk's AG
                    shS, rbS = shard_rows(g_sh, NB - 1, LA)
                    nc.sync.dma_start(out=shS[rbS + 127:rbS + 128, :],
                                      in_=sentg[:1, :])
                if b1 == NB or int(LA.of_block[b1]) != c:
                    ag(g_sh, g_tab, c, LA)

            # ------------- stage 2: GAT aggregate + u/v prep ---------------
            # hoisted softmax padding correction for all blocks:
            # cor = npad * exp(0.2*a_dst - 3.1)
            nc.vector.tensor_scalar(
                out=cor_all[:], in0=adst_all[:], scalar1=0.2, scalar2=-3.1,
                op0=OP.mult, op1=OP.add)
            nc.scalar.activation(cor_all[:], cor_all[:], AF.Exp)
            nc.vector.tensor_tensor(
                out=cor_all[:].rearrange("p (b h) -> p b h", h=4),
                in0=cor_all[:].rearrange("p (b h) -> p b h", h=4),
                in1=npad_t[:, :, None].to_broadcast([128, NB, 4]), op=OP.mult)

            led = _Ledger()
            for (b0, b1, c) in groups1B:
                nbg = b1 - b0
                Sg = off1[b1] - off1[b0]
                gt = gth.tile([128, Sg * GW], fp8, tag="gath", name="gt")
                nc.gpsimd.indirect_dma_start(
                    out=gt[:], out_offset=None, in_=g_tab[:, :],
                    in_offset=bass.IndirectOffsetOnAxis(
                        ap=idx1_t[:, off1[b0]:off1[b1]], axis=0))
                led.charge_pool(gat_gen_ns(Sg))
                ustg = stg.tile([128, nbg * H], fp8, tag="ustg", name="ustg")
                for b in range(b0, b1):
                    K = K1[b]
                    assert K >= 2
                    base = off1[b] - off1[b0]
                    g3 = gt[:, base * GW:(base + K) * GW].rearrange(
                        "p (k w) -> p k w", k=K)
                    # attention logits e = lrelu(a_src + a_dst), exp
                    et = wrk.tile([128, K * HEADS], bf16, tag="et", name="et")
                    e3 = et[:].rearrange("p (k h) -> p k h", k=K)
                    nc.vector.tensor_tensor(
                        out=e3, in0=g3[:, :, H:],
                        in1=adst_all[:, 4 * b:4 * b + 4][:, None, :]
                        .to_broadcast([128, K, HEADS]), op=OP.add)
                    nc.vector.scalar_tensor_tensor(
                        out=et[:], in0=et[:], scalar=0.2, in1=et[:],
                        op0=OP.mult, op1=OP.max)
                    led.charge_dve(K * HEADS * 2.1 + 340)
                    nc.scalar.activation(et[:], et[:], AF.Exp)
                    # weight g rows by exp(e) per head, then fold sums
                    wtf = wrk.tile([128, K * H], bf16, tag="kbuf",
                                   name="wtf")
                    w3 = wtf[:].rearrange("p (k d) -> p k d", k=K)
                    g4 = g3[:, :, :H].rearrange("p k (h c) -> p k h c",
                                                h=HEADS)
                    w4 = w3.rearrange("p k (h c) -> p k h c", h=HEADS)
                    e4 = e3[:, :, :, None].to_broadcast([128, K, HEADS, C])
                    e = led.pick(K * H, fast=False)
                    weng = nc.vector if e == "v" else nc.gpsimd
                    weng.tensor_tensor(out=w4, in0=g4, in1=e4, op=OP.mult)
                    _tree(nc, w3, K, nc.vector)
                    led.charge_dve((K - 1) * H * 0.52 + 850)
                    _tree(nc, e3, K, nc.vector)
                    led.charge_dve(K * HEADS * 0.52 + 500)
                    # denominator with hoisted padding correction
                    den = wrk.tile([128, HEADS], f32, name="den")
                    nc.vector.scalar_tensor_tensor(
                        out=den[:], in0=cor_all[:, 4 * b:4 * b + 4],
                        scalar=-1.0, in1=et[:, :HEADS],
                        op0=OP.mult, op1=OP.add)
                    rd = wrk.tile([128, HEADS], f32, name="rd")
                    nc.vector.reciprocal(rd[:], den[:])
                    h2 = wrk.tile([128, H], bf16, name="h2")
                    h2v = h2[:].rearrange("p (h c) -> p h c", h=HEADS)
                    nc.vector.tensor_tensor(
                        out=h2v,
                        in0=wtf[:, :H].rearrange("p (h c) -> p h c", h=HEADS),
                        in1=rd[:][:, :, None].to_broadcast([128, HEADS, C]),
                        op=OP.mult)
                    nc.vector.tensor_tensor(out=h2[:], in0=h2[:],
                                            in1=WB("gatb_bc"), op=OP.add)
                    led.charge_dve(2 * H * 1.04 + 500)
                    # elu + 1 (the -1 is folded into ubvb)
                    ng = wrk.tile([128, H], bf16, name="ng")
                    nc.vector.tensor_scalar_min(ng[:], h2[:], 0.0)
                    nc.scalar.activation(ng[:], ng[:], AF.Exp)
                    nc.vector.scalar_tensor_tensor(
                        out=h2[:], in0=h2[:], scalar=0.0, in1=ng[:],
                        op0=OP.max, op1=OP.add)
                    led.charge_dve(2 * H * 0.3 + 340)
                    h2T = transpose_bf(h2[:], "h2T")
                    pu = psmm.tile([128, 2 * H], f32, tag="mm", name="pu")
                    nc.tensor.matmul(out=pu[:], lhsT=h2T[:], rhs=WB("W1bd"),
                                     start=True, stop=False)
                    nc.tensor.matmul(out=pu[:], lhsT=onesb_row[:1, :],
                                     rhs=WB("ubvb")[:1, :], start=False,
                                     stop=True)
                    j = b - b0
                    nc.scalar.activation(ustg[:, j * H:(j + 1) * H],
                                         pu[:, :H], AF.Copy)
                    nc.scalar.activation(v_all[:, b * H:(b + 1) * H],
                                         pu[:, H:], AF.Copy)
                sh, rb = shard_rows(u_sh, b0, LB)
                nc.sync.dma_start(
                    out=sh[rb:rb + nbg * 128, :].rearrange(
                        "(j p) w -> p j w", p=128),
                    in_=ustg[:].rearrange("p (j w) -> p j w", j=nbg))
                if b1 == NB:
                    shS, rbS = shard_rows(u_sh, NB - 1, LB)
                    nc.sync.dma_start(out=shS[rbS + 127:rbS + 128, :],
                                      in_=sentu[:1, :])
                if b1 == NB or int(LB.of_block[b1]) != c:
                    ag(u_sh, u_tab, c, LB)

            # ---------------- stage 3: EdgeConv ----------------------------
            led = _Ledger()
            for (b0, b1, c) in groups2B:
                nbg = b1 - b0
                Sg = off2[b1] - off2[b0]
                ut = gth.tile([128, max(Sg, 1) * H], fp8, tag="gath",
                              name="ut")
                if Sg > 0:
                    nc.gpsimd.indirect_dma_start(
                        out=ut[:, :Sg * H], out_offset=None, in_=u_tab[:, :],
                        in_offset=bass.IndirectOffsetOnAxis(
                            ap=idx2e_t[:, off2[b0]:off2[b1]], axis=0))
                    led.charge_pool(gat_gen_ns(Sg))
                hstg = stg.tile([128, nbg * H], fp8, tag="hstg", name="hstg")
                for b in range(b0, b1):
                    K = K2[b]
                    base = off2[b] - off2[b0]
                    assert K > 0
                    u3 = ut[:, base * H:(base + K) * H].rearrange(
                        "p (k d) -> p k d", k=K)
                    uf = wrk.tile([128, K * H], bf16, tag="kbuf", name="uf")
                    u3f = uf[:].rearrange("p (k d) -> p k d", k=K)
                    e = led.pick(K * H, fast=False)
                    aeng = nc.vector if e == "v" else nc.gpsimd
                    aeng.tensor_tensor(
                        out=u3f, in0=u3,
                        in1=v_all[:, b * H:(b + 1) * H][:, None, :]
                        .to_broadcast([128, K, H]), op=OP.add)
                    G4 = (K + 3) // 4
                    red = wrk.tile([128, G4 * 128], bf16, tag="red",
                                   name="red")
                    k0 = 0
                    gi = 0
                    while k0 < K:
                        q = min(4, K - k0)
                        pt = ptbp.tile([128, q * 128], bf16, tag="ptb",
                                       name="ec_pt")
                        for j in range(q):
                            nc.tensor.transpose(
                                out=pt[:, j * 128:(j + 1) * 128],
                                in_=u3f[:, k0 + j, :], identity=identb[:])
                        m1 = wrk.tile([128, q * 128], bf16, tag="m1",
                                      name="m1")
                        nc.scalar.activation(m1[:], pt[:], AF.Relu)
                        pm = pstr.tile([128, q * 128], f32, tag="tr",
                                        name="ec_pm")
                        nc.tensor.matmul(out=pm[:], lhsT=WB("ec_W2"),
                                         rhs=m1[:], start=True, stop=True)
                        if q > 1:
                            nc.vector.tensor_reduce(
                                out=red[:, gi * 128:(gi + 1) * 128].rearrange(
                                    "p (n o) -> p n o", o=1),
                                in_=pm[:].rearrange(
                                    "p (j n) -> p n j", j=q),
                                axis=mybir.AxisListType.X, op=OP.max)
                            led.charge_dve(q * 128 * 1.04 + 300)
                        else:
                            nc.vector.tensor_copy(
                                out=red[:, gi * 128:(gi + 1) * 128],
                                in_=pm[:, :128])
                            led.charge_dve(128 * 1.04 + 300)
                        k0 += q
                        gi += 1
                    h3T = wrk.tile([128, H], bf16, name="h3T")
                    if G4 > 1:
                        accT = wrk.tile([128, H], bf16, name="accT")
                        nc.vector.tensor_reduce(
                            out=accT[:].rearrange("p (n o) -> p n o", o=1),
                            in_=red[:].rearrange("p (g n) -> p n g", g=G4),
                            axis=mybir.AxisListType.X, op=OP.max)
                        led.charge_dve(G4 * 128 * 1.04 + 170)
                        nc.scalar.activation(h3T[:], accT[:], AF.Relu,
                                             bias=WF("ecb2_c")[:, :1])
                    else:
                        nc.scalar.activation(h3T[:], red[:, :128], AF.Relu,
                                             bias=WF("ecb2_c")[:, :1])
                    ph3 = psmm.tile([128, 128], bf16, tag="mm", name="ph3")
                    nc.tensor.transpose(out=ph3[:], in_=h3T[:],
                                        identity=identb[:])
                    j = b - b0
                    nc.scalar.activation(hstg[:, j * H:(j + 1) * H], ph3[:],
                                         AF.Copy)
                    nc.vector.tensor_copy(
                        out=h3self[:, b * H:(b + 1) * H], in_=ph3[:])
                    led.charge_dve(H * 1.04 + 170)
                sh, rb = shard_rows(h3_sh, b0, LB)
                nc.sync.dma_start(
                    out=sh[rb:rb + nbg * 128, :].rearrange(
                        "(j p) w -> p j w", p=128),
                    in_=hstg[:].rearrange("p (j w) -> p j w", j=nbg))
                if b1 == NB:
                    shS, rbS = shard_rows(h3_sh, NB - 1, LB)
                    nc.sync.dma_start(out=shS[rbS + 127:rbS + 128, :],
                                      in_=zrow8[:1, :])
                if b1 == NB or int(LB.of_block[b1]) != c:
                    ag(h3_sh, h3_tab, c, LB)

            # ---------------- stage 4: GIN + gated pooling -----------------
            led = _Ledger()
            ppd = psacc.tile([G, H + 1], f32, tag="acc", name="ppd")
            bi = 0  # running block index for the accumulate chain
            for (b0, b1, c) in groups2B:
                Sg = off2[b1] - off2[b0]
                st = gth.tile([128, max(Sg, 1) * H], fp8, tag="gath",
                              name="st")
                if Sg > 0:
                    nc.gpsimd.indirect_dma_start(
                        out=st[:, :Sg * H], out_offset=None, in_=h3_tab[:, :],
                        in_offset=bass.IndirectOffsetOnAxis(
                            ap=idx2g_t[:, off2[b0]:off2[b1]], axis=0))
                    led.charge_pool(gat_gen_ns(Sg))
                for b in range(b0, b1):
                    K = K2[b]
                    base = off2[b] - off2[b0]
                    s = wrk.tile([128, H], bf16, name="s")
                    if K > 0:
                        s3 = st[:, base * H:(base + K) * H].rearrange(
                            "p (k d) -> p k d", k=K)
                        fold = wrk.tile([128, ((K + 1) // 2) * H], bf16,
                                        tag="fold", name="fold4")
                        e = led.pick((K // 2) * H, fast=False)
                        led.charge_dve((K - K // 2 - 1) * H * 0.52 + 340)
                        ssum = _fold_sum(nc, s3, fold, K, H,
                                         nc.vector if e == "v" else nc.gpsimd)
                        nc.vector.tensor_tensor(
                            out=s[:], in0=ssum,
                            in1=h3self[:, b * H:(b + 1) * H], op=OP.add)
                        led.charge_dve(H * 0.52 + 170)
                    else:
                        nc.vector.tensor_copy(
                            out=s[:], in_=h3self[:, b * H:(b + 1) * H])
                    sT = transpose_bf(s[:], "sT")
                    p1 = psmm.tile([128, 128], f32, tag="mm", name="p1")
                    nc.tensor.matmul(out=p1[:], lhsT=WB("gin_W1"),
                                     rhs=sT[:], start=True, stop=True)
                    t1 = wrk.tile([128, 128], bf16, name="t1")
                    nc.scalar.activation(t1[:], p1[:], AF.Relu,
                                         bias=WF("ginb1_c")[:, :1])
                    p2 = psmm.tile([128, H], f32, tag="mm", name="p2")
                    nc.tensor.matmul(out=p2[:], lhsT=WB("gin_W2"),
                                     rhs=t1[:], start=True, stop=True)
                    h4T = wrk.tile([128, H], bf16, name="h4T")
                    nc.scalar.activation(h4T[:], p2[:], AF.Relu,
                                         bias=WF("ginb2_c")[:, :1])
                    pg1 = psmm.tile([128, 128], f32, tag="mm", name="pg1")
                    nc.tensor.matmul(out=pg1[:], lhsT=WB("gate_W1"),
                                     rhs=h4T[:], start=True, stop=True)
                    g1 = wrk.tile([128, 128], bf16, name="g1")
                    nc.scalar.activation(g1[:], pg1[:], AF.Relu,
                                         bias=WF("gateb1_c")[:, :1])
                    pg2 = psa.tile([1, 128], f32, tag="psA", name="pg2")
                    nc.tensor.matmul(out=pg2[:], lhsT=WB("gate_W2"),
                                     rhs=g1[:], start=True, stop=True)
                    egT = wrk.tile([1, 128], f32, name="egT")
                    nc.scalar.activation(egT[:], pg2[:], AF.Exp,
                                         bias=gb2_t[:1, :1])
                    ph4 = psmm.tile([128, 128], bf16, tag="mm", name="ph4")
                    nc.tensor.transpose(out=ph4[:], in_=h4T[:],
                                        identity=identb[:])
                    h4r = wrk.tile([128, H + 1], f32, name="h4r")
                    nc.scalar.activation(h4r[:, :H], ph4[:], AF.Copy)
                    nc.vector.memset(h4r[:, H:H + 1], 1.0)
                    pe = psa.tile([128, 1], f32, tag="psA", name="pe")
                    nc.tensor.transpose(out=pe[:], in_=egT[:1, :],
                                        identity=ident[:1, :1])
                    oh = wrk.tile([128, G], f32, name="oh")
                    nc.vector.scalar_tensor_tensor(
                        out=oh[:], in0=WF("iota64"),
                        scalar=bids_t[:, b:b + 1],
                        in1=pe[:, :1].to_broadcast([128, G]),
                        op0=OP.is_equal, op1=OP.mult)
                    led.charge_dve(G * 1.04 + 170)
                    nc.tensor.matmul(
                        out=ppd[:, :H + 1], lhsT=oh[:], rhs=h4r[:],
                        start=(bi == 0), stop=(bi == NB - 1))
                    bi += 1

            # pooled partials -> AllReduce -> local head (baseline pattern)
            pl = cst.tile([G, 132], f32, name="pl")
            nc.vector.memset(pl[:], 0.0)
            nc.vector.tensor_copy(out=pl[:, :H + 1], in_=ppd[:])
            nc.sync.dma_start(out=ar_in[:, :], in_=pl[:])
            nc.gpsimd.collective_compute(
                "AllReduce", OP.add, ins=[ar_in[:, :].opt()],
                outs=[ar_out[:, :].opt()], replica_groups=REPL)
            ard = cst.tile([G, 132], f32, name="ard")
            nc.sync.dma_start(out=ard[:], in_=ar_out[:, :])
            rdn = cst.tile([G, 1], f32, name="rdn")
            nc.vector.reciprocal(rdn[:], ard[:, H:H + 1])
            pooled = cst.tile([G, H], f32, name="pooled")
            nc.vector.tensor_scalar_mul(pooled[:], ard[:, :H], rdn[:, :1])
            ppT = psmm.tile([128, G], f32, tag="mm", name="ppT")
            nc.tensor.transpose(out=ppT[:], in_=pooled[:G, :],
                                identity=ident[:G, :G])
            plT = cst.tile([128, G], f32, name="plT")
            nc.vector.tensor_copy(out=plT[:], in_=ppT[:])
            psl = psa.tile([G, OUT], f32, tag="psA", name="psl")
            nc.tensor.matmul(out=psl[:], lhsT=plT[:], rhs=WF("fc_W"),
                             start=True, stop=False)
            nc.tensor.matmul(out=psl[:], lhsT=ones_row[:1, :],
                             rhs=WF("fc_b")[:1, :], start=False, stop=True)
            rmx = cst.tile([G, 1], f32, name="rmx")
            nc.vector.tensor_reduce(out=rmx[:], in_=psl[:],
                                    axis=mybir.AxisListType.X, op=OP.max)
            xs = cst.tile([G, OUT], f32, name="xs")
            nc.vector.tensor_scalar(out=xs[:], in0=psl[:], scalar1=rmx[:, :1],
                                    scalar2=None, op0=OP.subtract)
            ex = cst.tile([G, OUT], f32, name="ex")
            ssum = cst.tile([G, 1], f32, name="ssum")
            nc.scalar.activation(ex[:], xs[:], AF.Exp, accum_out=ssum[:, :1])
            lg = cst.tile([G, 1], f32, name="lg")
            nc.scalar.activation(lg[:], ssum[:], AF.Ln)
            fin = cst.tile([G, OUT], f32, name="fin")
            nc.vector.tensor_scalar(out=fin[:], in0=xs[:], scalar1=lg[:, :1],
                                    scalar2=None, op0=OP.subtract)
            nc.sync.dma_start(out=outP[:, :], in_=fin[:])

    nc.compile()
    return nc


_CACHE = {}


def kernel(**inputs) -> np.ndarray:
    per_core, meta = _preprocess(**inputs)
    key = (tuple(meta["K1"]), tuple(meta["K2"]))
    if key not in _CACHE:
        _CACHE[key] = _build(meta)
    nc = _CACHE[key]
    res = run_bass_kernel_spmd(nc, per_core, list(range(R)))
    return np.asarray(res.results[0]["out"], np.float32)


if __name__ == "__main__":
    import reference
    inputs = {k: np.asarray(v) for k, v in reference.setup_inputs().items()}
    got = kernel(**inputs)
    print(got[:4])
